# revision 3
# baseline (speedup 1.0000x reference)
"""AdaptiveRankingLoss on 8 Trainium2 NeuronCores (Bass/Tile).

Math
----
reference:  loss = sum_{i<j, |t_i-t_j|>=0.05} 0.5*(w_i+w_j)*relu(-sign(td)*pd + m) / count
            td = t_i - t_j, pd = p_i - p_j, m = ms*0.08*clip(|td|, 0.1, 1.0)

Every per-pair factor (validity v, violation viol, margin m) is symmetric in
i<->j and the diagonal is invalid, so
    upper_sum / upper_count = full_sum / full_count
and by symmetry  sum_ij (w_i+w_j)*g_ij = 2 * sum_i w_i * rowsum_i(g),
with g = viol * v.  So each core computes full rows (no triangle mask), reduces
each row, dots the row-sums with w, and counts valid pairs; the host combines
8 (S, C) pairs:  loss = S_total / C_total.

Per [128 x F] tile (rows i on partitions, cols j on free dim):
    ACT: ad = Abs( tq_j - tq_i )        tq = bf16(0.08*ms*t)  -> ad = 0.08*ms*|td|
    ACT: s  = Sign( tq_i - tq_j )       in {-1, 0, +1}
    DVE: m  = (ad max lo) min hi        lo = 0.008*ms, hi = 0.08*ms   == margin
    DVE: v  = (ad is_ge theta)          theta = 0.004*ms  (<=> |td| >= 0.05)
    DVE: q  = (pq_j - pq_i) * s         scalar_tensor_tensor, == -sign(td)*pd
    DVE: vp = q + m
    DVE: g  = (vp max 0) * v            scalar_tensor_tensor, accum_out -> rowsum(g)
v's rowsum rides on accum_out as well; tiny f32 matmuls (lhsT=w / ones) reduce
the [128,1] row-sums across partitions into PSUM accumulators.

All t/p values are bf16-quantized identically on host for row-scalars and
broadcast tensors so every per-pair quantity stays exactly symmetric.
"""

import sys

if "/opt/trn_rl_repo" not in sys.path:
    sys.path.insert(0, "/opt/trn_rl_repo")

import numpy as np
import ml_dtypes

N = 8192
P = 128
N_CORES = 8
ROWS_PER_CORE = N // N_CORES          # 1024
BLOCKS = ROWS_PER_CORE // P           # 8 row blocks of 128 per core
F_CHUNK = 4096                        # free-dim chunk for compute tiles
N_CHUNKS = N // F_CHUNK               # 2
BCAST_CHUNK = 2048                    # free-dim chunk for broadcast DMAs
# which chunk instances compute v on ACT (Sign+Relu) instead of DVE, to
# balance engine load; index = block * N_CHUNKS + chunk
V_ON_ACT = set()

_CACHE = {}


def _build():
    from contextlib import ExitStack
    from concourse import bacc, tile, mybir

    BF16 = mybir.dt.bfloat16
    F32 = mybir.dt.float32
    Alu = mybir.AluOpType
    Act = mybir.ActivationFunctionType

    nc = bacc.Bacc("TRN2", target_bir_lowering=False, debug=False,
                   num_devices=N_CORES)

    tq_ext = nc.dram_tensor("tq", [1, N], BF16, kind="ExternalInput").ap()
    pq_ext = nc.dram_tensor("pq", [1, N], BF16, kind="ExternalInput").ap()
    ti_ext = nc.dram_tensor("ti", [P, BLOCKS], F32, kind="ExternalInput").ap()
    nti_ext = nc.dram_tensor("nti", [P, BLOCKS], F32, kind="ExternalInput").ap()
    pi_ext = nc.dram_tensor("pi", [P, BLOCKS], F32, kind="ExternalInput").ap()
    wi_ext = nc.dram_tensor("wi", [P, BLOCKS], F32, kind="ExternalInput").ap()
    # cst columns: 0=theta, 1=lo, 2=hi, 3=-theta
    cst_ext = nc.dram_tensor("cst", [P, 4], F32, kind="ExternalInput").ap()
    out_ext = nc.dram_tensor("out", [1, 2], F32, kind="ExternalOutput").ap()

    with tile.TileContext(nc) as tc:
        with ExitStack() as ctx:
            singles = ctx.enter_context(tc.tile_pool(name="singles", bufs=1))
            work = ctx.enter_context(tc.tile_pool(name="work", bufs=2))
            accp = ctx.enter_context(tc.tile_pool(name="accp", bufs=2))
            psum = ctx.enter_context(tc.tile_pool(name="psum", bufs=1, space="PSUM"))

            tqb = singles.tile([P, N], BF16)
            pqb = singles.tile([P, N], BF16)
            for c0 in range(0, N, BCAST_CHUNK):
                sl = slice(c0, c0 + BCAST_CHUNK)
                nc.sync.dma_start(out=tqb[:, sl],
                                  in_=tq_ext[:, sl].to_broadcast([P, BCAST_CHUNK]))
                nc.sync.dma_start(out=pqb[:, sl],
                                  in_=pq_ext[:, sl].to_broadcast([P, BCAST_CHUNK]))

            ti_sb = singles.tile([P, BLOCKS], F32)
            nc.sync.dma_start(out=ti_sb[:], in_=ti_ext[:])
            nti_sb = singles.tile([P, BLOCKS], F32)
            nc.sync.dma_start(out=nti_sb[:], in_=nti_ext[:])
            pi_sb = singles.tile([P, BLOCKS], F32)
            nc.sync.dma_start(out=pi_sb[:], in_=pi_ext[:])
            wi_sb = singles.tile([P, BLOCKS], F32)
            nc.sync.dma_start(out=wi_sb[:], in_=wi_ext[:])
            cst_sb = singles.tile([P, 4], F32)
            nc.sync.dma_start(out=cst_sb[:], in_=cst_ext[:])
            ones_sb = singles.tile([P, 1], F32)
            nc.gpsimd.memset(ones_sb[:], 1.0)

            ps_S = psum.tile([1, 1], F32)
            ps_C = psum.tile([1, 1], F32)

            for b in range(BLOCKS):
                accg = accp.tile([P, N_CHUNKS], F32, tag="accg")
                accv = accp.tile([P, N_CHUNKS], F32, tag="accv")
                for c in range(N_CHUNKS):
                    sl = slice(c * F_CHUNK, (c + 1) * F_CHUNK)
                    ad = work.tile([P, F_CHUNK], BF16, tag="ad")
                    nc.scalar.activation(out=ad[:], in_=tqb[:, sl], func=Act.Abs,
                                         bias=nti_sb[:, b:b + 1], scale=1.0)
                    s = work.tile([P, F_CHUNK], BF16, tag="s")
                    nc.scalar.activation(out=s[:], in_=tqb[:, sl], func=Act.Sign,
                                         bias=ti_sb[:, b:b + 1], scale=-1.0)
                    m = work.tile([P, F_CHUNK], BF16, tag="m")
                    nc.vector.tensor_scalar(
                        out=m[:], in0=ad[:],
                        scalar1=cst_sb[:, 1:2], scalar2=cst_sb[:, 2:3],
                        op0=Alu.max, op1=Alu.min)
                    v = work.tile([P, F_CHUNK], BF16, tag="v")
                    if b * N_CHUNKS + c in V_ON_ACT:
                        sv = work.tile([P, F_CHUNK], BF16, tag="sv")
                        nc.scalar.activation(out=sv[:], in_=ad[:], func=Act.Sign,
                                             bias=cst_sb[:, 3:4], scale=1.0)
                        nc.scalar.activation(out=v[:], in_=sv[:], func=Act.Relu,
                                             accum_out=accv[:, c:c + 1])
                    else:
                        nc.vector.tensor_scalar(
                            out=v[:], in0=ad[:],
                            scalar1=cst_sb[:, 0:1], scalar2=0.0,
                            op0=Alu.is_ge, op1=Alu.add,
                            accum_out=accv[:, c:c + 1])
                    q = work.tile([P, F_CHUNK], BF16, tag="q")
                    nc.vector.scalar_tensor_tensor(
                        out=q[:], in0=pqb[:, sl], scalar=pi_sb[:, b:b + 1],
                        in1=s[:], op0=Alu.subtract, op1=Alu.mult)
                    vp = work.tile([P, F_CHUNK], BF16, tag="vp")
                    nc.vector.tensor_tensor(out=vp[:], in0=q[:], in1=m[:],
                                            op=Alu.add)
                    g = work.tile([P, F_CHUNK], BF16, tag="g")
                    nc.vector.scalar_tensor_tensor(
                        out=g[:], in0=vp[:], scalar=0.0, in1=v[:],
                        op0=Alu.max, op1=Alu.mult,
                        accum_out=accg[:, c:c + 1])

                rowg = accp.tile([P, 1], F32, tag="rowg")
                nc.vector.tensor_reduce(out=rowg[:], in_=accg[:],
                                        axis=mybir.AxisListType.X, op=Alu.add)
                rowv = accp.tile([P, 1], F32, tag="rowv")
                nc.vector.tensor_reduce(out=rowv[:], in_=accv[:],
                                        axis=mybir.AxisListType.X, op=Alu.add)
                nc.tensor.matmul(ps_S[:], lhsT=wi_sb[:, b:b + 1], rhs=rowg[:],
                                 start=(b == 0), stop=(b == BLOCKS - 1))
                nc.tensor.matmul(ps_C[:], lhsT=ones_sb[:], rhs=rowv[:],
                                 start=(b == 0), stop=(b == BLOCKS - 1))

            out_sb = singles.tile([1, 2], F32)
            nc.vector.tensor_copy(out=out_sb[0:1, 0:1], in_=ps_S[:])
            nc.vector.tensor_copy(out=out_sb[0:1, 1:2], in_=ps_C[:])
            nc.sync.dma_start(out=out_ext[:], in_=out_sb[:])

    nc.compile()
    return nc


def _get_nc():
    if "nc" not in _CACHE:
        _CACHE["nc"] = _build()
    return _CACHE["nc"]


def _prepare_in_maps(predictions, targets, snr_weights, margin_scale):
    ms = float(margin_scale)
    bf16 = ml_dtypes.bfloat16

    t = np.asarray(targets, np.float32)
    p = np.asarray(predictions, np.float32)
    w = np.asarray(snr_weights, np.float32)

    # bf16-quantize once; use the SAME quantized values for broadcast tensors
    # and per-partition row scalars so pairwise terms stay exactly symmetric.
    tq = (0.08 * ms * t).astype(bf16)
    pq = p.astype(bf16)
    tqf = tq.astype(np.float32)
    pqf = pq.astype(np.float32)

    theta = np.float32(0.05 * 0.08 * ms)
    lo = np.float32(0.1 * 0.08 * ms)
    hi = np.float32(1.0 * 0.08 * ms)
    cst = np.zeros((P, 4), np.float32)
    cst[:, 0] = theta
    cst[:, 1] = lo
    cst[:, 2] = hi
    cst[:, 3] = -theta

    tq2 = tq.reshape(1, N)
    pq2 = pq.reshape(1, N)

    in_maps = []
    for core in range(N_CORES):
        r0 = core * ROWS_PER_CORE
        rows = slice(r0, r0 + ROWS_PER_CORE)
        # [ROWS_PER_CORE] -> [BLOCKS, P] -> [P, BLOCKS]
        ti = tqf[rows].reshape(BLOCKS, P).T.copy()
        pi = pqf[rows].reshape(BLOCKS, P).T.copy()
        wi = w[rows].reshape(BLOCKS, P).T.copy()
        in_maps.append({
            "tq": tq2,
            "pq": pq2,
            "ti": ti,
            "nti": -ti,
            "pi": pi,
            "wi": wi,
            "cst": cst,
        })
    return in_maps


def kernel(predictions, targets, snr_weights, margin_scale):
    from concourse.bass_utils import run_bass_kernel_spmd

    nc = _get_nc()
    in_maps = _prepare_in_maps(predictions, targets, snr_weights, margin_scale)
    res = run_bass_kernel_spmd(nc, in_maps, core_ids=list(range(N_CORES)))

    S = 0.0
    C = 0.0
    for r in res.results:
        S += float(r["out"][0, 0])
        C += float(r["out"][0, 1])
    loss = S / C if C > 0 else 0.0
    return np.float32(loss)


# revision 6
# speedup vs baseline: 1.9820x; 1.9820x over previous
"""AdaptiveRankingLoss on 8 Trainium2 NeuronCores (Bass/Tile), upper-triangle v2.

Math
----
reference:  loss = sum_{i<j, |t_i-t_j|>=0.05} 0.5*(w_i+w_j)*relu(-sign(td)*pd + m) / count
            td = t_i - t_j, pd = p_i - p_j, m = ms*0.08*clip(|td|, 0.1, 1.0)

Every per-pair factor is symmetric in i<->j, so each unordered pair needs to be
computed once.  Partition the 64x64 grid of 128-row blocks with a circulant
schedule: row-block I processes column-blocks J in the wrapped window
[I, I+n_I) mod 64, n_I = 33 for I<=31 and 32 for I>=32.  Every unordered block
pair lands in exactly one window (pair {I,J}, d=J-I: d<=32 -> I, else J), the
diagonal block leads each window (strict-upper 128x128 mask applied there), and
out-degrees are exactly {33 x 32 blocks, 32 x 32 blocks}.  Core k owns blocks
{4k..4k+3} (33-windows) and {32+4k..32+4k+3} (32-windows padded to 33 with
sentinel columns) -> all cores run identical shapes (SPMD) with identical work.

Per block (rows on partitions, window cols on free dim), all tensors bf16:
    ACT: ad  = Abs( tq_j - tq_i )     tq = bf16(0.08*ms*t); ad = 0.08*ms*|td|
    ACT: s   = Sign( tq_i - tq_j )
    DVE: m   = (ad max lo) min hi     == margin (lo=0.008ms, hi=0.08ms)
    DVE: v   = (ad is_ge theta)       theta = 0.004ms  (<=> |td| >= 0.05)
    DVE: pd  = pq_j - pq_i            (tensor_scalar)   [movable to ACT]
    DVE: q   = pd * s
    DVE: vp  = q + m
    ACT: viol= Relu(vp)                                 [movable to DVE]
    DVE: pw  = (wc_j + w_i) * 0.5     pair weight
    DVE: vw  = v * pw                 (diag cols further masked by U)
    DVE: gw  = viol * vw
PE reduces gw and v over partitions with ones-lhsT matmuls into two [1,512]
PSUM accumulators; host sums those in f64: loss = S / (C - pad_count).

Sentinel pad columns (slots 4..7): tq=10 -> v=1 (host subtracts the exact pad
count), pq=1e4 -> q=-1e4 -> viol=0 -> no effect on S.

All t/p values are bf16-quantized identically on host for row scalars and
column data so pairwise terms stay exactly symmetric.
"""

import sys

if "/opt/trn_rl_repo" not in sys.path:
    sys.path.insert(0, "/opt/trn_rl_repo")

import numpy as np
import ml_dtypes

N = 8192
P = 128
N_CORES = 8
NBLOCKS_TOTAL = N // P                 # 64 row blocks globally
SLOTS = 8                              # row blocks per core
W_BLK = 33                             # column-blocks per window (incl pad)
W = W_BLK * P                          # 4224 window columns
PAD_SLOTS = (4, 5, 6, 7)               # slots with 128 sentinel pad columns
T_PAD = 10.0
P_PAD = 1.0e4
# blocks slots where Relu(vp) runs on DVE instead of ACT (load-balance knob)
RELU_ON_DVE = set()
# block slots where pd runs on ACT instead of DVE (load-balance knob)
PD_ON_ACT = {0, 2, 4, 6}

_CACHE = {}


def _core_blocks(core):
    return [4 * core + i for i in range(4)] + [32 + 4 * core + i for i in range(4)]


def _window(I):
    n = W_BLK if I <= 31 else W_BLK - 1
    return [(I + j) % NBLOCKS_TOTAL for j in range(n)]


def _build():
    from contextlib import ExitStack
    from concourse import bacc, tile, mybir

    BF16 = mybir.dt.bfloat16
    F32 = mybir.dt.float32
    Alu = mybir.AluOpType
    Act = mybir.ActivationFunctionType

    nc = bacc.Bacc("TRN2", target_bir_lowering=False, debug=False,
                   num_devices=N_CORES)

    tqc_ext = nc.dram_tensor("tqc", [SLOTS, W], BF16, kind="ExternalInput").ap()
    pqc_ext = nc.dram_tensor("pqc", [SLOTS, W], BF16, kind="ExternalInput").ap()
    wc_ext = nc.dram_tensor("wc", [SLOTS, W], BF16, kind="ExternalInput").ap()
    ti_ext = nc.dram_tensor("ti", [P, SLOTS], F32, kind="ExternalInput").ap()
    nti_ext = nc.dram_tensor("nti", [P, SLOTS], F32, kind="ExternalInput").ap()
    pi_ext = nc.dram_tensor("pi", [P, SLOTS], F32, kind="ExternalInput").ap()
    npi_ext = nc.dram_tensor("npi", [P, SLOTS], F32, kind="ExternalInput").ap()
    wi_ext = nc.dram_tensor("wi", [P, SLOTS], F32, kind="ExternalInput").ap()
    um_ext = nc.dram_tensor("um", [P, P], BF16, kind="ExternalInput").ap()
    # cst columns: 0=theta, 1=lo, 2=hi
    cst_ext = nc.dram_tensor("cst", [P, 4], F32, kind="ExternalInput").ap()
    out_ext = nc.dram_tensor("out", [1, 1024], F32, kind="ExternalOutput").ap()

    with tile.TileContext(nc) as tc:
        with ExitStack() as ctx:
            singles = ctx.enter_context(tc.tile_pool(name="singles", bufs=1))
            bcast = ctx.enter_context(tc.tile_pool(name="bcast", bufs=2))
            work = ctx.enter_context(tc.tile_pool(name="work", bufs=10))
            small = ctx.enter_context(tc.tile_pool(name="small", bufs=4))
            psum = ctx.enter_context(tc.tile_pool(name="psum", bufs=1, space="PSUM"))

            ti_sb = singles.tile([P, SLOTS], F32)
            nc.sync.dma_start(out=ti_sb[:], in_=ti_ext[:])
            nti_sb = singles.tile([P, SLOTS], F32)
            nc.sync.dma_start(out=nti_sb[:], in_=nti_ext[:])
            pi_sb = singles.tile([P, SLOTS], F32)
            nc.sync.dma_start(out=pi_sb[:], in_=pi_ext[:])
            npi_sb = singles.tile([P, SLOTS], F32)
            nc.sync.dma_start(out=npi_sb[:], in_=npi_ext[:])
            wi_sb = singles.tile([P, SLOTS], F32)
            nc.sync.dma_start(out=wi_sb[:], in_=wi_ext[:])
            um_sb = singles.tile([P, P], BF16)
            nc.sync.dma_start(out=um_sb[:], in_=um_ext[:])
            cst_sb = singles.tile([P, 4], F32)
            nc.sync.dma_start(out=cst_sb[:], in_=cst_ext[:])
            ones_sb = singles.tile([P, 1], BF16)
            nc.gpsimd.memset(ones_sb[:], 1.0)

            ps_S = psum.tile([1, 512], F32)
            ps_C = psum.tile([1, 512], F32)

            HALF = W // 2  # broadcast DMA split for queue parallelism
            first_mm = [True]

            for b in range(SLOTS):
                tq_bc = bcast.tile([P, W], BF16, tag="tq_bc")
                pq_bc = bcast.tile([P, W], BF16, tag="pq_bc")
                wc_bc = bcast.tile([P, W], BF16, tag="wc_bc")
                for lo_, hi_ in ((0, HALF), (HALF, W)):
                    nc.sync.dma_start(
                        out=tq_bc[:, lo_:hi_],
                        in_=tqc_ext[b:b + 1, lo_:hi_].to_broadcast([P, hi_ - lo_]))
                    nc.sync.dma_start(
                        out=pq_bc[:, lo_:hi_],
                        in_=pqc_ext[b:b + 1, lo_:hi_].to_broadcast([P, hi_ - lo_]))
                    nc.sync.dma_start(
                        out=wc_bc[:, lo_:hi_],
                        in_=wc_ext[b:b + 1, lo_:hi_].to_broadcast([P, hi_ - lo_]))

                ad = work.tile([P, W], BF16, tag="wk")
                nc.scalar.activation(out=ad[:], in_=tq_bc[:], func=Act.Abs,
                                     bias=nti_sb[:, b:b + 1], scale=1.0)
                s = work.tile([P, W], BF16, tag="wk")
                nc.scalar.activation(out=s[:], in_=tq_bc[:], func=Act.Sign,
                                     bias=ti_sb[:, b:b + 1], scale=-1.0)
                m = work.tile([P, W], BF16, tag="wk")
                nc.vector.tensor_scalar(
                    out=m[:], in0=ad[:],
                    scalar1=cst_sb[:, 1:2], scalar2=cst_sb[:, 2:3],
                    op0=Alu.max, op1=Alu.min)
                v = work.tile([P, W], BF16, tag="wk")
                nc.vector.tensor_scalar(
                    out=v[:], in0=ad[:], scalar1=cst_sb[:, 0:1], scalar2=None,
                    op0=Alu.is_ge)
                pd = work.tile([P, W], BF16, tag="wk")
                if b in PD_ON_ACT:
                    nc.scalar.activation(out=pd[:], in_=pq_bc[:],
                                         func=Act.Identity,
                                         bias=npi_sb[:, b:b + 1], scale=1.0)
                else:
                    nc.vector.tensor_scalar(
                        out=pd[:], in0=pq_bc[:], scalar1=pi_sb[:, b:b + 1],
                        scalar2=None, op0=Alu.subtract)
                q = work.tile([P, W], BF16, tag="wk")
                nc.vector.tensor_tensor(out=q[:], in0=pd[:], in1=s[:],
                                        op=Alu.mult)
                vp = work.tile([P, W], BF16, tag="wk")
                nc.vector.tensor_tensor(out=vp[:], in0=q[:], in1=m[:],
                                        op=Alu.add)
                pw = work.tile([P, W], BF16, tag="wk")
                nc.vector.tensor_scalar(
                    out=pw[:], in0=wc_bc[:], scalar1=wi_sb[:, b:b + 1],
                    scalar2=0.5, op0=Alu.add, op1=Alu.mult)
                # strict-upper mask for the leading diagonal block
                vm = small.tile([P, P], BF16, tag="vm")
                nc.vector.tensor_tensor(out=vm[:], in0=v[:, 0:P], in1=um_sb[:],
                                        op=Alu.mult)
                vw = work.tile([P, W], BF16, tag="wk")
                nc.vector.tensor_tensor(out=vw[:, 0:P], in0=vm[:],
                                        in1=pw[:, 0:P], op=Alu.mult)
                nc.vector.tensor_tensor(out=vw[:, P:W], in0=v[:, P:W],
                                        in1=pw[:, P:W], op=Alu.mult)
                gw = work.tile([P, W], BF16, tag="wk")
                if b in RELU_ON_DVE:
                    h = work.tile([P, W], BF16, tag="wk")
                    nc.vector.tensor_tensor(out=h[:], in0=vp[:], in1=vw[:],
                                            op=Alu.mult)
                    nc.vector.tensor_scalar(
                        out=gw[:], in0=h[:], scalar1=0.0, scalar2=None,
                        op0=Alu.max)
                else:
                    viol = work.tile([P, W], BF16, tag="wk")
                    nc.scalar.activation(out=viol[:], in_=vp[:], func=Act.Relu)
                    nc.vector.tensor_tensor(out=gw[:], in0=viol[:], in1=vw[:],
                                            op=Alu.mult)

                # PE reductions over partitions into PSUM accumulators.
                # start=True only on the first full-width matmul per
                # accumulator (it overwrites the whole [1,512] region);
                # everything else accumulates.
                last = (b == SLOTS - 1)
                # gw: 8x512 + 1x128 chunks
                for ci in range(8):
                    f0 = ci * 512
                    nc.tensor.matmul(ps_S[:, 0:512], lhsT=ones_sb[:],
                                     rhs=gw[:, f0:f0 + 512],
                                     start=(b == 0 and ci == 0), stop=False)
                nc.tensor.matmul(ps_S[:, 0:P], lhsT=ones_sb[:],
                                 rhs=gw[:, 8 * 512:W],
                                 start=False, stop=last)
                # v: one full-width 512 chunk first (start resets), then the
                # 128-wide vm, then the remaining chunks
                nc.tensor.matmul(ps_C[:, 0:512], lhsT=ones_sb[:],
                                 rhs=v[:, P:P + 512],
                                 start=(b == 0), stop=False)
                nc.tensor.matmul(ps_C[:, 0:P], lhsT=ones_sb[:], rhs=vm[:],
                                 start=False, stop=False)
                for ci in range(1, 8):
                    f0 = P + ci * 512
                    nc.tensor.matmul(ps_C[:, 0:512], lhsT=ones_sb[:],
                                     rhs=v[:, f0:f0 + 512],
                                     start=False, stop=(last and ci == 7))

            out_sb = singles.tile([1, 1024], F32)
            nc.scalar.copy(out=out_sb[0:1, 0:512], in_=ps_S[:])
            nc.scalar.copy(out=out_sb[0:1, 512:1024], in_=ps_C[:])
            nc.sync.dma_start(out=out_ext[:], in_=out_sb[:])

    nc.compile()
    return nc


def _get_nc():
    if "nc" not in _CACHE:
        _CACHE["nc"] = _build()
    return _CACHE["nc"]


def _prepare_in_maps(predictions, targets, snr_weights, margin_scale):
    ms = float(margin_scale)
    bf16 = ml_dtypes.bfloat16

    t = np.asarray(targets, np.float32)
    p = np.asarray(predictions, np.float32)
    w = np.asarray(snr_weights, np.float32)

    # bf16-quantize once; identical values feed column data and row scalars so
    # every pairwise term is exactly symmetric.
    tq = (0.08 * ms * t).astype(bf16)
    pq = p.astype(bf16)
    wq = w.astype(bf16)
    tqf = tq.astype(np.float32)
    pqf = pq.astype(np.float32)
    wqf = wq.astype(np.float32)

    cst = np.zeros((P, 4), np.float32)
    cst[:, 0] = np.float32(0.05 * 0.08 * ms)
    cst[:, 1] = np.float32(0.1 * 0.08 * ms)
    cst[:, 2] = np.float32(1.0 * 0.08 * ms)

    um = np.triu(np.ones((P, P), np.float32), k=1).astype(bf16)

    tq_blocks = tq.reshape(NBLOCKS_TOTAL, P)
    pq_blocks = pq.reshape(NBLOCKS_TOTAL, P)
    wq_blocks = wq.reshape(NBLOCKS_TOTAL, P)

    in_maps = []
    for core in range(N_CORES):
        blocks = _core_blocks(core)
        tqc = np.full((SLOTS, W), T_PAD, bf16)
        pqc = np.full((SLOTS, W), P_PAD, bf16)
        wc = np.zeros((SLOTS, W), bf16)
        ti = np.empty((P, SLOTS), np.float32)
        pi = np.empty((P, SLOTS), np.float32)
        wi = np.empty((P, SLOTS), np.float32)
        for slot, I in enumerate(blocks):
            win = _window(I)
            nw = len(win) * P
            tqc[slot, :nw] = tq_blocks[win].reshape(-1)
            pqc[slot, :nw] = pq_blocks[win].reshape(-1)
            wc[slot, :nw] = wq_blocks[win].reshape(-1)
            rows = slice(I * P, (I + 1) * P)
            ti[:, slot] = tqf[rows]
            pi[:, slot] = pqf[rows]
            wi[:, slot] = wqf[rows]
        in_maps.append({
            "tqc": tqc, "pqc": pqc, "wc": wc,
            "ti": ti, "nti": -ti, "pi": pi, "npi": -pi, "wi": wi,
            "um": um, "cst": cst,
        })
    return in_maps


def kernel(predictions, targets, snr_weights, margin_scale):
    from concourse.bass_utils import run_bass_kernel_spmd

    nc = _get_nc()
    in_maps = _prepare_in_maps(predictions, targets, snr_weights, margin_scale)
    res = run_bass_kernel_spmd(nc, in_maps, core_ids=list(range(N_CORES)))

    S = 0.0
    C = 0.0
    for r in res.results:
        S += float(np.asarray(r["out"][0, 0:512], np.float64).sum())
        C += float(np.asarray(r["out"][0, 512:1024], np.float64).sum())
    C -= float(N_CORES * len(PAD_SLOTS) * P * P)  # sentinel pad columns
    loss = S / C if C > 0 else 0.0
    return np.float32(loss)
